# revision 2
# baseline (speedup 1.0000x reference)
"""Block-sparse linear y = x @ W^T on 8 Trainium2 NeuronCores.

Strategy: densify W^T on the host (the 32x32 block scatter is not exploitable
on a 128x128 PE array) and run a dense GEMM sharded 4-way over tokens x 2-way
over out_features (no collectives). The GEMM runs in fp8 (e4m3) with the PE's
DoubleRow mode, which contracts two 128-k subtiles per instruction at twice
the bf16/fp32r MAC rate. Full precision is recovered with a 3-pass split:
  x = xh + xl, W = wh + wl (each fp8),  y ~= xh@wh + xl@wh + xh@wl
(the dropped xl@wl term is O(2^-8) relative; measured rel err ~1e-3 vs the
fp32 reference, 20x under the 2e-2 gate). PE cost is 3 fp8-DoubleRow passes
= 0.75x the single bf16/fp32r GEMM; input DMA halves vs fp32 (8MB/core), and
y drains as bf16 (2MB/core), converted back to fp32 on the host.

Schedule per core, two phases by token-half so 8 PSUM banks cover
(4 m-groups x 2 n-tiles) and each stationary x tile feeds both n-tile
streams back-to-back (hides LDWEIGHTS): phase mh: for each k-pair kp,
run series (xh@wh, xl@wh, xh@wl) over m in 0..3, n in 0..1, accumulating
24 DoubleRow matmuls per bank, then drain psum -> bf16 -> y. Inputs are
host-packed into exact SBUF layouts ([P, KT, free], 2-4KB contiguous runs)
and ride the sync DMA queue in consumption order; y drains ride the scalar
queue. A few fp32 warmup matmuls keep the PE clock gate hot during the DMA
head wait.
"""

import numpy as np

TOKENS, IN_F, OUT_F = 4096, 2048, 2048
BLOCK = 32
N_CORES = 8
TG, OG = 4, 2  # token groups x out-feature groups
T_SH = TOKENS // TG  # 1024 tokens per core
O_SH = OUT_F // OG  # 1024 out features per core
P = 128
NFREE = 512  # PSUM bank free dim (fp32)
KT = IN_F // P  # 16 k subtiles
KP = KT // 2  # 8 DoubleRow k-pairs
TH = T_SH // 2  # 512-token halves (phases)
MH = TH // P  # 4 m-groups per phase
NT = O_SH // NFREE  # 2 out column tiles
CHUNK = 4  # k-subtiles per input DMA chunk

TRACE = False  # set by test.py to capture an NTFF profile
MM_DTYPE = "float8"  # informational; test.py --fp32 sets this but is unused

_nc_cache = {}
_last_result = None  # BassKernelResults of the most recent run (for test.py)


def _build_nc():
    import concourse.mybir as mybir
    import concourse.tile as tile
    from concourse import bacc

    key = "fp8x3"
    if key in _nc_cache:
        return _nc_cache[key]

    f8 = mybir.dt.float8e4
    f32 = mybir.dt.float32
    bf16 = mybir.dt.bfloat16
    DR = mybir.MatmulPerfMode.DoubleRow

    nc = bacc.Bacc(None, target_bir_lowering=False)
    # Host-pre-packed inputs (exact SBUF layouts; all DMAs are linear):
    # xh/xl: x^T hi/lo fp8 by token-half, [2][P][KT][TH]
    # wh/wl: W^T hi/lo fp8, [P][KT][O_SH]
    xh_d = nc.dram_tensor("xh", [2, P, KT, TH], f8, kind="ExternalInput")
    xl_d = nc.dram_tensor("xl", [2, P, KT, TH], f8, kind="ExternalInput")
    wh_d = nc.dram_tensor("wh", [P, KT, O_SH], f8, kind="ExternalInput")
    wl_d = nc.dram_tensor("wl", [P, KT, O_SH], f8, kind="ExternalInput")
    y = nc.dram_tensor("y", [T_SH, O_SH], bf16, kind="ExternalOutput")

    with tile.TileContext(nc) as tc:
        with (
            tc.tile_pool(name="xp", bufs=1) as xp,
            tc.tile_pool(name="wp", bufs=1) as wp,
            tc.tile_pool(name="op", bufs=8) as op,
            tc.tile_pool(name="ps", bufs=1, space="PSUM") as ps,
        ):
            # Warm the PE's HAM clock gate during the initial DMA wait: fp32
            # dummy matmuls take the array past the 3.4us busy window so the
            # first real matmuls run at 2.4GHz.
            zt = xp.tile([P, NFREE], f32, tag="warm", name="warm")
            nc.gpsimd.memset(zt[:], 0.0)
            warm_ps = ps.tile([P, NFREE], f32, tag="ps0", name="warm_ps")
            for _ in range(3):
                nc.tensor.matmul(warm_ps[:], zt[:, :P], zt[:], start=True, stop=True)

            # SBUF tiles: x by token-half (phase) and hi/lo; W hi/lo full.
            xs_t = [[None, None], [None, None]]  # [hi/lo][token-half]
            for h in range(2):
                xs_t[0][h] = xp.tile([P, KT, TH], f8, tag=f"xh{h}", name=f"xh{h}")
                xs_t[1][h] = xp.tile([P, KT, TH], f8, tag=f"xl{h}", name=f"xl{h}")
            wh_t = wp.tile([P, KT, O_SH], f8, tag="wh", name="wh")
            wl_t = wp.tile([P, KT, O_SH], f8, tag="wl", name="wl")

            # Input DMAs on the sync queue, in consumption order: phase-0
            # x chunks + W chunks interleaved, then phase-1 x chunks.
            for c in range(KT // CHUNK):
                ck = slice(c * CHUNK, (c + 1) * CHUNK)
                nc.sync.dma_start(xs_t[0][0][:, ck, :], xh_d[0, :, ck, :])
                nc.sync.dma_start(wh_t[:, ck, :], wh_d[:, ck, :])
                nc.sync.dma_start(xs_t[1][0][:, ck, :], xl_d[0, :, ck, :])
                nc.sync.dma_start(wl_t[:, ck, :], wl_d[:, ck, :])
            for c in range(KT // CHUNK):
                ck = slice(c * CHUNK, (c + 1) * CHUNK)
                nc.sync.dma_start(xs_t[0][1][:, ck, :], xh_d[1, :, ck, :])
                nc.sync.dma_start(xs_t[1][1][:, ck, :], xl_d[1, :, ck, :])

            for mh in range(2):  # token-half phases
                psums = [
                    ps.tile([P, NFREE], f32, tag=f"ps{b}", name=f"ps{b}")
                    for b in range(8)
                ]
                # 3-pass fp8 series: hi@hi, lo@hi, hi@lo (xl@wl dropped).
                series = [
                    (xs_t[0][mh], wh_t),
                    (xs_t[1][mh], wh_t),
                    (xs_t[0][mh], wl_t),
                ]
                for kp in range(KP):
                    kk = slice(2 * kp, 2 * kp + 2)
                    for si, (xt, wt) in enumerate(series):
                        for m in range(MH):
                            lhs = xt[:, kk, m * P : (m + 1) * P]
                            for n in range(NT):
                                nc.tensor.matmul(
                                    psums[m * NT + n][:],
                                    lhs,
                                    wt[:, kk, n * NFREE : (n + 1) * NFREE],
                                    start=(kp == 0 and si == 0),
                                    stop=(kp == KP - 1 and si == 2),
                                    perf_mode=DR,
                                )
                for m in range(MH):  # drain as bf16 on the scalar queue
                    for n in range(NT):
                        ot = op.tile([P, NFREE], bf16, tag="ot")
                        nc.vector.tensor_copy(ot[:], psums[m * NT + n][:])
                        row = (mh * MH + m) * P
                        nc.scalar.dma_start(
                            y[row : row + P, n * NFREE : (n + 1) * NFREE], ot[:]
                        )

    nc.compile()
    _nc_cache[key] = nc
    return nc


def _densify_wT(weight_blocks, block_rows, block_cols):
    """Scatter-add the 32x32 blocks into dense W^T [in_features, out_features]."""
    nc_blk = IN_F // BLOCK
    nr_blk = OUT_F // BLOCK
    wcr = np.zeros((nc_blk, nr_blk, BLOCK, BLOCK), np.float32)
    # block b occupies W[32r:32r+32, 32c:32c+32]; W^T gets the transposed block
    np.add.at(
        wcr,
        (block_cols.astype(np.int64), block_rows.astype(np.int64)),
        np.swapaxes(weight_blocks.astype(np.float32, copy=False), 1, 2),
    )
    return np.ascontiguousarray(wcr.transpose(0, 2, 1, 3).reshape(IN_F, OUT_F))


def _split_fp8(a):
    """Split fp32 array into (hi, lo) e4m3 parts with hi + lo ~= a."""
    import ml_dtypes

    f8 = ml_dtypes.float8_e4m3
    hi = a.astype(f8)
    lo = (a - hi.astype(np.float32)).astype(f8)
    return hi, lo


def _pack_core_inputs(xT_sh, wT_sh):
    """Split one core's x^T / W^T shards into fp8 hi/lo DMA layouts."""
    out = {}
    xh, xl = _split_fp8(xT_sh)
    for name, arr in (("xh", xh), ("xl", xl)):
        # [2048, 1024] -> [kt, p, h, t] -> [h, p, kt, t]
        out[name] = np.ascontiguousarray(
            arr.reshape(KT, P, 2, TH).transpose(2, 1, 0, 3)
        )
    wh, wl = _split_fp8(wT_sh)
    for name, arr in (("wh", wh), ("wl", wl)):
        # [2048, 1024] -> [kt, p, o] -> [p, kt, o]
        out[name] = np.ascontiguousarray(arr.reshape(KT, P, O_SH).transpose(1, 0, 2))
    return out


def kernel(x, weight_blocks, block_rows, block_cols):
    global _last_result
    from concourse.bass_utils import run_bass_kernel_spmd

    x = np.asarray(x, dtype=np.float32)
    wT = _densify_wT(
        np.asarray(weight_blocks), np.asarray(block_rows), np.asarray(block_cols)
    )
    xT = np.ascontiguousarray(x.T)

    in_maps = []
    for c in range(N_CORES):
        tg, og = divmod(c, OG)
        in_maps.append(
            _pack_core_inputs(
                xT[:, tg * T_SH : (tg + 1) * T_SH],
                wT[:, og * O_SH : (og + 1) * O_SH],
            )
        )

    nc = _build_nc()
    res = None
    for attempt in range(3):  # transient NRT device errors happen; retry
        try:
            res = run_bass_kernel_spmd(
                nc, in_maps, core_ids=list(range(N_CORES)), trace=TRACE
            )
            break
        except Exception:
            if attempt == 2:
                raise
            import time

            time.sleep(3)
    _last_result = res

    y = np.empty((TOKENS, OUT_F), np.float32)
    for c in range(N_CORES):
        tg, og = divmod(c, OG)
        y[tg * T_SH : (tg + 1) * T_SH, og * O_SH : (og + 1) * O_SH] = (
            res.results[c]["y"].astype(np.float32)
        )
    return y


# revision 3
# speedup vs baseline: 1.4296x; 1.4296x over previous
"""Block-sparse linear y = x @ W^T on 8 Trainium2 NeuronCores.

Strategy: densify W^T on the host (the 32x32 block scatter is not exploitable
on a 128x128 PE array) and run a dense bf16 GEMM, sharded 4-way over tokens x
2-way over out_features (no collectives). bf16 streams at the PE's full rate
(1 cycle/row, same as fp32r) while halving input DMA vs fp32: 8MB in + 2MB
y out (bf16, converted to fp32 on the host) per core, ~58.6us of matmul
stream vs a ~28us DMA floor, so the kernel is PE-stream-bound with ~2x DMA
slack and none of the fp32 baseline's staging stalls. (fp8 DoubleRow was
measured: 2x MACs/instruction but the same 1 cycle/row stream, so the 3-pass
hi/lo-split fp8 GEMM needs 1.5x the instructions = strictly worse; bf16's
2e-3 rel err has 10x margin on the 2e-2 gate.)

Schedule per core, two phases by token-half so 8 PSUM banks cover
(4 m-groups x 2 n-tiles) and each stationary x tile feeds both n-tile
streams back-to-back (hides LDWEIGHTS). Phase 0 runs k-outer with x/W
chunks staged just-in-time; phase 1 (data resident) runs per-m k-inner so
each m-pair of banks drains the moment it finishes, shrinking the tail.
Inputs are host-packed into exact SBUF layouts ([P, KT, free], 2-8KB
contiguous runs) and ride the sync DMA queue in consumption order; y drains
ride the scalar queue. A few fp32 warmup matmuls keep the PE clock gate hot
during the DMA head wait.
"""

import numpy as np

TOKENS, IN_F, OUT_F = 4096, 2048, 2048
BLOCK = 32
N_CORES = 8
TG, OG = 4, 2  # token groups x out-feature groups
T_SH = TOKENS // TG  # 1024 tokens per core
O_SH = OUT_F // OG  # 1024 out features per core
P = 128
NFREE = 512  # PSUM bank free dim (fp32)
KT = IN_F // P  # 16 k subtiles
TH = T_SH // 2  # 512-token halves (phases)
MH = TH // P  # 4 m-groups per phase
NT = O_SH // NFREE  # 2 out column tiles
CHUNK = 2  # k-subtiles per input DMA chunk

TRACE = False  # set by test.py to capture an NTFF profile
MM_DTYPE = "bfloat16"  # informational; test.py --fp32 sets this but is unused

_nc_cache = {}
_last_result = None  # BassKernelResults of the most recent run (for test.py)


def _build_nc():
    import concourse.mybir as mybir
    import concourse.tile as tile
    from concourse import bacc

    key = "bf16"
    if key in _nc_cache:
        return _nc_cache[key]

    f32 = mybir.dt.float32
    bf16 = mybir.dt.bfloat16

    nc = bacc.Bacc(None, target_bir_lowering=False)
    # Host-pre-packed inputs (exact SBUF layouts; all DMAs are linear):
    # x: x^T bf16 by token-half, [2][P][KT][TH]; w: W^T bf16, [P][KT][O_SH]
    x_d = nc.dram_tensor("x", [2, P, KT, TH], bf16, kind="ExternalInput")
    w_d = nc.dram_tensor("w", [P, KT, O_SH], bf16, kind="ExternalInput")
    y = nc.dram_tensor("y", [T_SH, O_SH], bf16, kind="ExternalOutput")

    with tile.TileContext(nc) as tc:
        with (
            tc.tile_pool(name="xp", bufs=1) as xp,
            tc.tile_pool(name="wp", bufs=1) as wp,
            tc.tile_pool(name="op", bufs=8) as op,
            tc.tile_pool(name="ps", bufs=1, space="PSUM") as ps,
        ):
            # Warm the PE's HAM clock gate during the initial DMA wait: fp32
            # dummy matmuls take the array past the 3.4us busy window so the
            # first real matmuls run at 2.4GHz.
            zt = xp.tile([P, NFREE], f32, tag="warm", name="warm")
            nc.gpsimd.memset(zt[:], 0.0)
            warm_ps = ps.tile([P, NFREE], f32, tag="ps0", name="warm_ps")
            for _ in range(3):
                nc.tensor.matmul(warm_ps[:], zt[:, :P], zt[:], start=True, stop=True)

            x_t = [
                xp.tile([P, KT, TH], bf16, tag=f"x{h}", name=f"x{h}")
                for h in range(2)
            ]
            w_t = wp.tile([P, KT, O_SH], bf16, tag="w", name="w")

            # Input DMAs on the sync queue in consumption order: phase-0 x
            # and W chunks interleaved, then phase-1 x chunks.
            for c in range(KT // CHUNK):
                ck = slice(c * CHUNK, (c + 1) * CHUNK)
                nc.sync.dma_start(x_t[0][:, ck, :], x_d[0, :, ck, :])
                nc.sync.dma_start(w_t[:, ck, :], w_d[:, ck, :])
            for c in range(KT // CHUNK):
                ck = slice(c * CHUNK, (c + 1) * CHUNK)
                nc.sync.dma_start(x_t[1][:, ck, :], x_d[1, :, ck, :])

            def bank(m, n):
                return ps.tile([P, NFREE], f32, tag=f"ps{m * NT + n}", name=f"ps{m}{n}")

            def drain(m, n, psum, mh):
                ot = op.tile([P, NFREE], bf16, tag="ot")
                nc.vector.tensor_copy(ot[:], psum[:])
                row = (mh * MH + m) * P
                nc.scalar.dma_start(
                    y[row : row + P, n * NFREE : (n + 1) * NFREE], ot[:]
                )

            # ---- Phase 0 (token-half 0): k-outer, chunks staged JIT ----
            psums = [[bank(m, n) for n in range(NT)] for m in range(MH)]
            for k in range(KT):
                for m in range(MH):
                    lhs = x_t[0][:, k, m * P : (m + 1) * P]
                    for n in range(NT):
                        nc.tensor.matmul(
                            psums[m][n][:],
                            lhs,
                            w_t[:, k, n * NFREE : (n + 1) * NFREE],
                            start=(k == 0),
                            stop=(k == KT - 1),
                        )
            for m in range(MH):
                for n in range(NT):
                    drain(m, n, psums[m][n], 0)

            # ---- Phase 1 (token-half 1): data resident; per-m k-inner so
            # each m-pair of banks drains as soon as it finishes. ----
            for m in range(MH):
                pb = [bank(m, n) for n in range(NT)]
                for k in range(KT):
                    lhs = x_t[1][:, k, m * P : (m + 1) * P]
                    for n in range(NT):
                        nc.tensor.matmul(
                            pb[n][:],
                            lhs,
                            w_t[:, k, n * NFREE : (n + 1) * NFREE],
                            start=(k == 0),
                            stop=(k == KT - 1),
                        )
                for n in range(NT):
                    drain(m, n, pb[n], 1)

    nc.compile()
    _nc_cache[key] = nc
    return nc


def _densify_wT(weight_blocks, block_rows, block_cols):
    """Scatter-add the 32x32 blocks into dense W^T [in_features, out_features]."""
    nc_blk = IN_F // BLOCK
    nr_blk = OUT_F // BLOCK
    wcr = np.zeros((nc_blk, nr_blk, BLOCK, BLOCK), np.float32)
    # block b occupies W[32r:32r+32, 32c:32c+32]; W^T gets the transposed block
    np.add.at(
        wcr,
        (block_cols.astype(np.int64), block_rows.astype(np.int64)),
        np.swapaxes(weight_blocks.astype(np.float32, copy=False), 1, 2),
    )
    return np.ascontiguousarray(wcr.transpose(0, 2, 1, 3).reshape(IN_F, OUT_F))


def _pack_core_inputs(xT_sh, wT_sh):
    """Cast one core's x^T / W^T shards to bf16 in the kernel's DMA layouts."""
    import ml_dtypes

    bf16 = ml_dtypes.bfloat16
    # [2048, 1024] -> [kt, p, h, t] -> [h, p, kt, t]
    x = np.ascontiguousarray(
        xT_sh.astype(bf16).reshape(KT, P, 2, TH).transpose(2, 1, 0, 3)
    )
    # [2048, 1024] -> [kt, p, o] -> [p, kt, o]
    w = np.ascontiguousarray(
        wT_sh.astype(bf16).reshape(KT, P, O_SH).transpose(1, 0, 2)
    )
    return {"x": x, "w": w}


def kernel(x, weight_blocks, block_rows, block_cols):
    global _last_result
    from concourse.bass_utils import run_bass_kernel_spmd

    x = np.asarray(x, dtype=np.float32)
    wT = _densify_wT(
        np.asarray(weight_blocks), np.asarray(block_rows), np.asarray(block_cols)
    )
    xT = np.ascontiguousarray(x.T)

    in_maps = []
    for c in range(N_CORES):
        tg, og = divmod(c, OG)
        in_maps.append(
            _pack_core_inputs(
                xT[:, tg * T_SH : (tg + 1) * T_SH],
                wT[:, og * O_SH : (og + 1) * O_SH],
            )
        )

    nc = _build_nc()
    res = None
    for attempt in range(3):  # transient NRT device errors happen; retry
        try:
            res = run_bass_kernel_spmd(
                nc, in_maps, core_ids=list(range(N_CORES)), trace=TRACE
            )
            break
        except Exception:
            if attempt == 2:
                raise
            import time

            time.sleep(3)
    _last_result = res

    y = np.empty((TOKENS, OUT_F), np.float32)
    for c in range(N_CORES):
        tg, og = divmod(c, OG)
        y[tg * T_SH : (tg + 1) * T_SH, og * O_SH : (og + 1) * O_SH] = (
            res.results[c]["y"].astype(np.float32)
        )
    return y
